# revision 2
# baseline (speedup 1.0000x reference)
"""Trainium2 Bass kernel v2 for the pre-norm transformer block (B=16,N=1024,C=768,H=12).

Data-parallel over batch, 2 batch elems per core. Single fused pipeline with
persistent pools so the Tile scheduler overlaps stages across batch elems:
QKV(b1) runs on PE/DVE while attention-exp(b0) grinds on ScalarE, and
MLP(b0) overlaps attention(b1).

vs baseline:
- LN gammas folded into wqkv/wfc1 host-side; LN betas applied via K=1 bias
  matmuls into the QKV/V/fc1 PSUM accumulations (general, ~10us PE).
- LN stats: single PE accumulation chain over [xb|xb^2] (two chains in one
  PSUM bank would clobber each other's has_written bits).
- exp at FD=1024 from [128,2,512] f32 PSUM score tiles (halves ACT op count).
- AV pair packed into one [128,512] PSUM bank (even head rows 0-63, odd
  64-127); softmax denominators via separate ones-lhsT DR chains; epilogue
  is pair-combined (1 rb copy + 1 o-mult per cq instead of per head).
- GPSIMD (Pool) offloads LN bf16 copies/squares and small row copies.
- Residual stream stays in SBUF f32 (no DRAM spill); q/k stored fp8.
- Weight/param DMAs hoisted out of the rep body where lifetimes allow
  (wqkv/wfc1 share one SBUF slot and reload per body to fit SBUF).
"""

import numpy as np
import ml_dtypes
from contextlib import ExitStack

import concourse.bass as bass
import concourse.tile as tile
import concourse.mybir as mybir
from concourse.bass_utils import run_bass_kernel_spmd
from concourse.mybir import AluOpType as alu
from concourse.mybir import ActivationFunctionType as act

F32 = mybir.dt.float32
BF16 = mybir.dt.bfloat16
FP8 = mybir.dt.float8e4
BF16_NP = ml_dtypes.bfloat16
FP8_NP = ml_dtypes.float8_e4m3
WS = 64.0
WSI = 1.0 / 64.0

B, N, C, H, HD, MLP = 16, 1024, 768, 12, 64, 3072
EPS = 1e-5
NCORES = 8
BPC = B // NCORES
T = BPC * N                # 2048 tokens per core
CK = 512
FT = C // 128              # 6
VT = MLP // 128            # 24
TKT = N // 128             # 8 key tiles per batch elem
DR = mybir.MatmulPerfMode.DoubleRow


def _patched_drain_and_barrier(self, tick_clock, wait_clock):
    # This walrus build rejects >2 sync waits on one Drain; spread the
    # end-of-kernel waits over single-wait NOPs.
    import bass_rust
    from concourse.vector_clock import ScopedClock

    drain_inst = self.nc.sync.drain()
    wait_clock.add_sem_waits(
        drain_inst.ins, ScopedClock({None: tick_clock.global_clock})
    )
    si = drain_inst.ins.sync_info
    waits = list(si.on_wait) if si is not None and si.on_wait else []
    if len(waits) > 1:
        si.on_wait = waits[:1]
        for w in waits[1:]:
            nop = self.nc.sync.nop(nofuse=True)
            nsi = nop.ins.sync_info
            if nsi is None:
                nop.ins.sync_info = bass_rust.SyncInfo(on_wait=[w], on_update=[])
            else:
                nsi.on_wait = [w]
    self.nc.all_engine_barrier()
    popped = self.nc._tile_sem_poison_stack.pop()
    assert popped is self._sem_poison
    self.nc.clear_and_free_semaphores(list(self.sems.allocated().values()))
    self.nc.all_engine_barrier()


tile.TileContext._drain_and_barrier = _patched_drain_and_barrier

_MAXW = 1


def _split_sync_waits(nc):
    """Move excess per-instruction sync waits onto same-engine NOPs."""
    import bass_rust

    nsplit = 0
    for bb in nc.m.functions[0].blocks:
        insts = bb.instructions
        i = 0
        while i < len(insts):
            inst = insts[i]
            si = inst.sync_info
            if si is not None and si.on_wait and len(si.on_wait) > _MAXW:
                waits = list(si.on_wait)
                si.on_wait = waits[:_MAXW]
                extra = waits[_MAXW:]
                pos = i
                for j in range(0, len(extra), _MAXW):
                    nop = mybir.InstNoOp(
                        name=f"{inst.name}_wsplit{j}",
                        engine=inst.engine,
                        bass_nofuse=True,
                        sync_info=bass_rust.SyncInfo(
                            on_wait=extra[j:j + _MAXW], on_update=[]),
                    )
                    insts.insert(pos, nop)
                    pos += 1
                    i += 1
                    nsplit += 1
            i += 1
    return nsplit


_CACHE = {}


def _build_program(reps=1):
    key = ("nc2", reps)
    if key in _CACHE:
        return _CACHE[key]
    nc = bass.Bass()

    x_d = nc.dram_tensor("xT", [128, FT, T], BF16, kind="ExternalInput")
    wqkv_d = nc.dram_tensor("wqkv", [128, FT, 3 * C], FP8, kind="ExternalInput")
    wproj_d = nc.dram_tensor("wproj", [128, FT, C], FP8, kind="ExternalInput")
    wfc1_d = nc.dram_tensor("wfc1", [128, FT, MLP], FP8, kind="ExternalInput")
    wfc2_d = nc.dram_tensor("wfc2", [128, VT, C], FP8, kind="ExternalInput")
    bls1_d = nc.dram_tensor("bls1", [128, FT], F32, kind="ExternalInput")
    bls2_d = nc.dram_tensor("bls2", [128, FT], F32, kind="ExternalInput")
    bfc1_d = nc.dram_tensor("bfc1", [128, VT], F32, kind="ExternalInput")
    # LN1 beta contributions: qkvb = feat-cols of ln1_b @ wqkv[:, :2C]
    # (added per-partition at q/k evac); vb = row of ln1_b @ wqkv_v x WS
    # (added via K=1 matmul into the V psum; V bias varies along the free
    # dim so a per-partition scalar cannot express it). ln2_b @ w_fc1 is
    # folded into bfc1 host-side (per-partition in gelu).
    qkvb_d = nc.dram_tensor("qkvb", [128, 2 * FT], F32, kind="ExternalInput")
    vb_d = nc.dram_tensor("vb", [1, C], BF16, kind="ExternalInput")
    out_d = nc.dram_tensor("outT", [128, FT, T], F32, kind="ExternalOutput")

    with tile.TileContext(nc) as tc, ExitStack() as ctx:
        const = ctx.enter_context(tc.tile_pool(name="const", bufs=1))
        params = ctx.enter_context(tc.tile_pool(name="params", bufs=1))
        wts = ctx.enter_context(tc.tile_pool(name="wts", bufs=1))
        big = ctx.enter_context(tc.tile_pool(name="bigsb", bufs=1))
        rows = ctx.enter_context(tc.tile_pool(name="rows", bufs=1))
        scr = ctx.enter_context(tc.tile_pool(name="scr", bufs=1))
        avsp = ctx.enter_context(tc.tile_pool(name="avsp", bufs=2))
        ypool = ctx.enter_context(tc.tile_pool(name="ypool", bufs=4))
        xbq_pool = ctx.enter_context(tc.tile_pool(name="xbq", bufs=2))
        outp = ctx.enter_context(tc.tile_pool(name="outp", bufs=3))
        epool = ctx.enter_context(tc.tile_pool(name="epool", bufs=1))
        hpool = ctx.enter_context(tc.tile_pool(name="hpool", bufs=1))
        ps_sc = ctx.enter_context(
            tc.tile_pool(name="ps_sc", bufs=3, space="PSUM"))
        ps_mm = ctx.enter_context(
            tc.tile_pool(name="ps_mm", bufs=2, space="PSUM"))

        ones_col = const.tile([128, 1], BF16)
        nc.vector.memset(ones_col, 1.0)
        ones_row = const.tile([1, 512], BF16)
        nc.vector.memset(ones_row, 1.0)
        eps_row = const.tile([1, 1], F32)
        nc.vector.memset(eps_row, EPS * WS * WS)

        bls1 = params.tile([128, FT], F32)
        nc.sync.dma_start(bls1, bls1_d[:, :])
        bls2 = params.tile([128, FT], F32)
        nc.sync.dma_start(bls2, bls2_d[:, :])
        bfc1 = params.tile([128, VT], F32)
        nc.sync.dma_start(bfc1, bfc1_d[:, :])
        qkvb = params.tile([128, 2 * FT], F32)
        nc.sync.dma_start(qkvb, qkvb_d[:, :])
        vb = params.tile([1, C], BF16)
        nc.sync.dma_start(vb, vb_d[:, :])

        # wqkv and wfc1 share one SBUF slot (disjoint lifetimes within a
        # body); wproj/wfc2 are persistent.
        wproj = wts.tile([128, FT, C], FP8, tag="wproj")
        wfc2 = wts.tile([128, VT, C], FP8, tag="wfc2")
        for ft in range(FT):
            nc.sync.dma_start(wproj[:, ft, :], wproj_d[:, ft, :])
            nc.sync.dma_start(wfc2[:, 4 * ft:4 * ft + 4, :],
                              wfc2_d[:, 4 * ft:4 * ft + 4, :])

        # persistent activation tensors, split per batch elem so cross-batch
        # stage overlap carries no false WAR dependencies
        x_b = [big.tile([128, FT, N], BF16, tag=f"x{b}", name=f"x{b}") for b in range(BPC)]
        q_b = [big.tile([128, FT, N], FP8, tag=f"q{b}", name=f"qt{b}") for b in range(BPC)]
        k_b = [big.tile([128, FT, N], FP8, tag=f"k{b}", name=f"kt{b}") for b in range(BPC)]
        v_b = [big.tile([128, TKT, H, 80], FP8, tag=f"v{b}", name=f"vt{b}") for b in range(BPC)]
        o_b = [big.tile([128, FT, N], FP8, tag=f"o{b}", name=f"ot{b}") for b in range(BPC)]
        for b in range(BPC):
            # ones column at [*,*,*,64] folds the softmax denominator into
            # AV; memset once (outside the rep loop), V evacs leave it intact
            nc.gpsimd.memset(v_b[b].rearrange("p a b c -> p (a b c)"), 1.0)

        y2_of = {}

        def emit_ln(ch, y_out):
            ctx = {}
            ln_stats(ch, ctx)
            ln_norm(ch, ctx, y_out)

        def ln_stats(ch, ctx):
            """LayerNorm stats for chunk ch -> rsnb16 row (rs, -mu*rs)."""
            x_sb = x_b[ch // 2]
            c0 = (ch % 2) * CK
            xbq = xbq_pool.tile([128, FT, 2, CK], BF16, tag="xbq")
            for ft in range(FT):
                nc.gpsimd.tensor_copy(xbq[:, ft, 0, :], x_sb[:, ft, c0:c0 + CK])
                nc.gpsimd.tensor_mul(xbq[:, ft, 1, :], xbq[:, ft, 0, :],
                                     xbq[:, ft, 0, :])
            ps_st = ps_sc.tile([1, 2, CK], F32, tag="sc", name=f"st{ch}")
            # two accumulation chains in two different banks of one tile
            # (a single 1024-wide fp32 matmul would span 2 banks: illegal)
            for ft in range(FT):
                for j in range(2):
                    nc.tensor.matmul(ps_st[:, j, :], lhsT=ones_col,
                                     rhs=xbq[:, ft, j, :],
                                     start=(ft == 0), stop=(ft == FT - 1))
            inv_c = 1.0 / C
            muex = rows.tile([1, 2, CK], F32, tag="muex")
            nc.vector.tensor_scalar_mul(muex.rearrange("p a b -> p (a b)"),
                                        ps_st.rearrange("p a b -> p (a b)"),
                                        inv_c)
            var = rows.tile([1, CK], F32, tag="var")
            nc.vector.scalar_tensor_tensor(
                var, in0=muex[:, 0, :], scalar=-1.0, in1=muex[:, 0, :],
                op0=alu.mult, op1=alu.mult)          # -mu^2
            nc.vector.tensor_tensor(var, muex[:, 1, :], var, alu.add)
            nc.scalar.activation(var, var, act.Sqrt, bias=eps_row)  # std
            nc.vector.reciprocal(var, var)               # var <- rs
            nc.vector.scalar_tensor_tensor(
                muex[:, 0, :], in0=muex[:, 0, :], scalar=-1.0, in1=var,
                op0=alu.mult, op1=alu.mult)          # muex0 <- -mu*rs
            rsnb16 = rows.tile([1, 2, CK], BF16, tag="rsnb16")
            nc.gpsimd.tensor_copy(rsnb16[:, 0, :], var)
            nc.gpsimd.tensor_copy(rsnb16[:, 1, :], muex[:, 0, :])
            ctx["rsnb16"] = rsnb16
            ctx["xbq"] = xbq

        def ln_norm(ch, ctx, y_out):
            """LayerNorm normalize for chunk ch from stats in ctx."""
            rsnb16 = ctx["rsnb16"]
            xbq = ctx["xbq"]
            ps_bc = ps_sc.tile([128, 2, CK], F32, tag="sc", name=f"bc{ch}")
            nc.tensor.matmul(ps_bc[:, 0, :], lhsT=ones_row[:, 0:128],
                             rhs=rsnb16[:, 0, :], start=True, stop=True)
            nc.tensor.matmul(ps_bc[:, 1, :], lhsT=ones_row[:, 0:128],
                             rhs=rsnb16[:, 1, :], start=True, stop=True)
            bcab = scr.tile([128, 2, CK], BF16, tag="bcab")
            nc.vector.tensor_copy(bcab.rearrange("p a b -> p (a b)"),
                                  ps_bc.rearrange("p a b -> p (a b)"))
            for ft in range(FT):
                t1 = scr.tile([128, CK], BF16, tag="t1")
                nc.vector.tensor_tensor(t1, xbq[:, ft, 0, :], bcab[:, 0, :],
                                        alu.mult)
                nc.vector.tensor_tensor(y_out[:, ft, :], t1, bcab[:, 1, :],
                                        alu.add)

        def qkv_pieces(ch, wqkv):
            """Return (ln_piece, qk_pieces[mt 0..11], v_pieces[8]) for
            chunk ch: q/k weight-stationary feature-major, V activation-
            stationary token-major. LN1 beta enters via a K=1 bias matmul
            for V; q/k betas are per-partition scalars at evac."""
            q_t, k_t, v_t = q_b[ch // 2], k_b[ch // 2], v_b[ch // 2]
            c0 = (ch % 2) * CK
            y1 = ypool.tile([128, FT, CK], FP8, tag="y", name=f"y1_{ch}")
            lctx = {}
            ln_st = lambda: ln_stats(ch, lctx)
            ln_piece = lambda: ln_norm(ch, lctx, y1)
            def qk_piece(mt):
                ps = ps_mm.tile([128, CK], F32, tag="mm", name=f"qk{ch}_{mt}")
                for kd in range(FT // 2):
                    nc.tensor.matmul(
                        ps,
                        lhsT=wqkv[:, 2 * kd:2 * kd + 2,
                                  mt * 128:(mt + 1) * 128],
                        rhs=y1[:, 2 * kd:2 * kd + 2, :],
                        perf_mode=DR, start=(kd == 0),
                        stop=(kd == FT // 2 - 1))
                dst = q_t if mt < FT else k_t
                nc.vector.tensor_scalar(
                    dst[:, mt % FT, c0:c0 + CK], ps,
                    scalar1=WSI, op0=alu.mult,
                    scalar2=qkvb[:, mt:mt + 1], op1=alu.add)
            qk_pieces = [lambda mt=mt: qk_piece(mt) for mt in range(2 * FT)]
            # V: activation-stationary -> token-major
            def v_piece(mtok, nv):
                gtok = (ch % 2) * (CK // 128) + mtok
                ps = ps_mm.tile([128, CK], F32, tag="mm",
                                name=f"v{ch}_{mtok}_{nv}")
                psv = ps[:, 0:384]
                for kd in range(FT // 2):
                    nc.tensor.matmul(
                        psv,
                        lhsT=y1[:, 2 * kd:2 * kd + 2,
                                mtok * 128:(mtok + 1) * 128],
                        rhs=wqkv[:, 2 * kd:2 * kd + 2,
                                 2 * C + nv * 384:2 * C + (nv + 1) * 384],
                        perf_mode=DR, start=(kd == 0), stop=False)
                nc.tensor.matmul(
                    psv, lhsT=ones_row[:, 0:128],
                    rhs=vb[:, nv * 384:(nv + 1) * 384],
                    start=False, stop=True)
                nc.vector.tensor_scalar_mul(
                    v_t[:, gtok, nv * 6:(nv + 1) * 6, 0:HD],
                    psv.rearrange("p (h d) -> p h d", h=6), WSI)
            v_pieces = [lambda mtok=mtok, nv=nv: v_piece(mtok, nv)
                        for nv in range(2) for mtok in range(CK // 128)]
            return ln_st, ln_piece, qk_pieces, v_pieces

        def attn_pieces(b, hp):
            """Head pair hp of batch elem b: scores (fp8 q/k, even/odd head
            in disjoint PE row groups), exp at FD=1024, denominator DR
            chains, pair-packed AV, pair-combined normalize."""
            heads = (2 * hp, 2 * hp + 1)
            q_t, k_t, v_t, o_t = q_b[b], k_b[b], v_b[b], o_b[b]
            e2 = {}
            for h in heads:
                e2[h] = epool.tile([128, TKT, N], FP8, tag=f"e{h % 2}",
                                   name=f"e2_{b}_{h}")
            def sc_piece(h, tkt):
                tk0 = tkt * 128
                fq, po = h // 2, (h % 2) * 64
                psc = ps_sc.tile([128, 2, CK], F32, tag="sc",
                                 name=f"sc{b}_{hp}_{tkt}_{h % 2}")
                for cq in range(2):
                    tq0 = cq * CK
                    nc.tensor.matmul(
                        psc[:, cq, :],
                        lhsT=k_t[po:po + 64, fq, tk0:tk0 + 128],
                        rhs=q_t[po:po + 64, fq, tq0:tq0 + CK],
                        start=True, stop=True)
                nc.scalar.activation(
                    e2[h][:, tkt, :],
                    psc.rearrange("p a b -> p (a b)"), act.Exp)
            sc_pieces = [lambda h=h, tkt=tkt: sc_piece(h, tkt)
                         for tkt in range(TKT) for h in heads]
            def av_piece(cq):
                tq0 = cq * CK
                ps_avs = {}
                for h in heads:
                    ps_avs[h] = ps_mm.tile([65, CK], F32, tag="mm",
                                            name=f"av{b}_{hp}_{cq}_{h % 2}")
                for tkd in range(TKT // 2):
                    for h in heads:
                        rhs = e2[h][:, 2 * tkd:2 * tkd + 2, cq * CK:cq * CK + CK]
                        nc.tensor.matmul(
                            ps_avs[h],
                            lhsT=v_t[:, 2 * tkd:2 * tkd + 2, h, 0:HD + 1],
                            rhs=rhs, perf_mode=DR,
                            start=(tkd == 0), stop=(tkd == TKT // 2 - 1))
                avs = scr.tile([65, 2, CK], F32, tag="avs")
                for h in heads:
                    nc.vector.tensor_copy(avs[:, h % 2, :], ps_avs[h])
                nc.vector.reciprocal(avs[64:65, :, :], avs[64:65, :, :])
                r216 = rows.tile([1, 2, CK], BF16, tag="r216")
                nc.gpsimd.tensor_copy(r216.rearrange("p a b -> p (a b)"),
                                      avs[64:65, :, :])
                ps_rb = ps_sc.tile([128, CK], F32, tag="sc",
                                    name=f"rb{b}_{hp}_{cq}")
                nc.tensor.matmul(ps_rb[0:64, :], lhsT=ones_row[:, 0:64],
                                 rhs=r216[:, 0, :], start=True, stop=True)
                nc.tensor.matmul(ps_rb[64:128, :], lhsT=ones_row[:, 0:64],
                                 rhs=r216[:, 1, :], start=True, stop=True)
                nc.vector.tensor_tensor(
                    o_t[0:64, hp, tq0:tq0 + CK], avs[0:64, 0, :],
                    ps_rb[0:64, :], alu.mult)
                nc.vector.tensor_tensor(
                    o_t[64:128, hp, tq0:tq0 + CK], avs[0:64, 1, :],
                    ps_rb[64:128, :], alu.mult)
            av_pieces = [lambda cq=cq: av_piece(cq) for cq in range(2)]
            return sc_pieces, av_pieces

        def projln_pieces(ch):
            """proj + residual1 (in-place into x_sb) + LN2 for chunk ch."""
            x_sb, o_t = x_b[ch // 2], o_b[ch // 2]
            c0 = (ch % 2) * CK
            def proj_piece(mt):
                ps = ps_mm.tile([128, CK], F32, tag="mm", name=f"pj{ch}_{mt}")
                for kd in range(FT // 2):
                    nc.tensor.matmul(
                        ps,
                        lhsT=wproj[:, 2 * kd:2 * kd + 2, mt * 128:(mt + 1) * 128],
                        rhs=o_t[:, 2 * kd:2 * kd + 2, c0:c0 + CK],
                        perf_mode=DR, start=(kd == 0), stop=(kd == FT // 2 - 1))
                nc.vector.scalar_tensor_tensor(
                    x_sb[:, mt, c0:c0 + CK], in0=ps,
                    scalar=bls1[:, mt:mt + 1], in1=x_sb[:, mt, c0:c0 + CK],
                    op0=alu.add, op1=alu.add)
            for mt in range(FT):
                yield lambda mt=mt: proj_piece(mt)
            y2 = ypool.tile([128, FT, CK], FP8, tag="y", name=f"y2_{ch}")
            yield lambda: emit_ln(ch, y2)
            y2_of[ch] = y2

        def mlp_pieces(ch, wfc1):
            """fc1/gelu + fc2 + residual2 + out DMA for chunk ch."""
            x_sb = x_b[ch // 2]
            c0 = (ch % 2) * CK
            y2 = y2_of[ch]
            h_t = hpool.tile([128, VT, CK], FP8, tag="h", name=f"h_{ch}")
            def fc1_piece(mt):
                ps = ps_mm.tile([128, CK], F32, tag="mm", name=f"f1{ch}_{mt}")
                for kd in range(FT // 2):
                    nc.tensor.matmul(
                        ps,
                        lhsT=wfc1[:, 2 * kd:2 * kd + 2,
                                  mt * 128:(mt + 1) * 128],
                        rhs=y2[:, 2 * kd:2 * kd + 2, :],
                        perf_mode=DR, start=(kd == 0),
                        stop=(kd == FT // 2 - 1))
                nc.scalar.activation(h_t[:, mt, :], ps, act.Gelu,
                                     bias=bfc1[:, mt:mt + 1], scale=WSI)
            for mt in range(VT):
                yield lambda mt=mt: fc1_piece(mt)
            def fc2_piece(mt):
                ps = ps_mm.tile([128, CK], F32, tag="mm", name=f"f2{ch}_{mt}")
                for kd in range(VT // 2):
                    nc.tensor.matmul(
                        ps,
                        lhsT=wfc2[:, 2 * kd:2 * kd + 2, mt * 128:(mt + 1) * 128],
                        rhs=h_t[:, 2 * kd:2 * kd + 2, :],
                        perf_mode=DR, start=(kd == 0), stop=(kd == VT // 2 - 1))
                o_fin = outp.tile([128, CK], F32, tag="of")
                nc.vector.scalar_tensor_tensor(
                    o_fin, in0=ps, scalar=bls2[:, mt:mt + 1],
                    in1=x_sb[:, mt, c0:c0 + CK], op0=alu.add, op1=alu.add)
                nc.sync.dma_start(out_d[:, mt, ch * CK:ch * CK + CK], o_fin)
            for mt in range(FT):
                yield lambda mt=mt: fc2_piece(mt)

        def zip_emit(primary, filler, clump=8):
            """Emit all pieces of `primary` with `filler` pieces spread
            between them in clumps of `clump` (larger clumps amortize ACT
            table-set switches when filler carries gelu/sqrt)."""
            if not filler:
                for p in primary:
                    p()
                return
            ratio = len(filler) / max(len(primary), 1)
            acc = 0.0
            fi = 0
            for p in primary:
                p()
                acc += ratio
                if acc >= clump:
                    while fi < len(filler) and acc >= 1.0:
                        filler[fi]()
                        fi += 1
                        acc -= 1.0
            while fi < len(filler):
                filler[fi]()
                fi += 1

        def ordered_qkv_fillers(chs, wqkv):
            """QK/V pieces of two chunks, ordered by head-pair need: q/k
            feature tiles for hp, then the V half consumed first."""
            st0, ln0, qk0, v0 = qkv_pieces(chs[0], wqkv)
            st1, ln1, qk1, v1 = qkv_pieces(chs[1], wqkv)
            st0()
            ln0()
            st1()
            ln1()
            out = []
            order = []
            for i in range(FT):
                order += [("qk", i), ("qk", i + FT)]
                if i == 0:
                    order += [("v", 0), ("v", 1), ("v", 2), ("v", 3)]
                if i == 2:
                    order += [("v", 4), ("v", 5), ("v", 6), ("v", 7)]
            for kind, idx in order:
                src_ = (qk0, qk1) if kind == "qk" else (v0, v1)
                out.append(src_[0][idx])
                out.append(src_[1][idx])
            return out

        def attn_pipelined(b):
            """Attention pieces for batch elem b, with each head pair's
            AV/epilogue delayed behind the next pair's score/exp stream so
            the epilogue chain never blocks the exp pacing."""
            pieces = []
            prev_av = []
            for hp in range(H // 2):
                sc, av = attn_pieces(b, hp)
                pieces += sc[0:2]
                rest = sc[2:]
                k = len(prev_av)
                for i, p in enumerate(rest):
                    pieces.append(p)
                    if i < k:
                        pieces.append(prev_av[i])
                prev_av = av
            pieces += prev_av
            return pieces

        def emit_body():
            for b in range(BPC):
                for ft in range(FT):
                    nc.sync.dma_start(x_b[b][:, ft, :],
                                      x_d[:, ft, b * N:(b + 1) * N])
            wqkv = wts.tile([128, FT, 3 * C], FP8, tag="wbig", name="wqkv")
            for ft in range(FT):
                nc.sync.dma_start(wqkv[:, ft, :], wqkv_d[:, ft, :])
            # phase 1: all four LN1s (sqrts batched, exp-free), then just
            # enough of QKV(b0) for head pair 0
            fill0 = ordered_qkv_fillers((0, 1), wqkv)
            fill1 = ordered_qkv_fillers((2, 3), wqkv)
            pre, rest0 = fill0[0:12], fill0[12:]
            for p in pre:
                p()
            # phase 2: attention(b0) paced by exp; remaining QKV(b0) then
            # QKV(b1) matmuls zip into the PE/DVE slack
            zip_emit(attn_pipelined(0), rest0 + fill1)
            wfc1 = wts.tile([128, FT, MLP], FP8, tag="wbig", name="wfc1")
            for ft in range(FT):
                nc.sync.dma_start(wfc1[:, ft, :], wfc1_d[:, ft, :])
            # phase 3: attention(b1) with proj+LN2+MLP(b0) zipped in clumps
            # (clumped so gelu/sqrt table-set switches amortize)
            mlp0 = [p for cq in range(2) for p in projln_pieces(cq)]
            mlp0 += [p for cq in range(2) for p in mlp_pieces(cq, wfc1)]
            zip_emit(attn_pipelined(1), mlp0, clump=8)
            # phase 4: proj+LN2+MLP of b1
            for cq in range(2):
                for p in projln_pieces(2 + cq):
                    p()
            for cq in range(2):
                for p in mlp_pieces(2 + cq, wfc1):
                    p()

        for _rep in range(reps):
            emit_body()

    _split_sync_waits(nc)
    _CACHE[key] = nc
    return nc


def _feat_cols(v):
    return np.ascontiguousarray(np.asarray(v, np.float32).reshape(-1, 128).T)


def make_in_maps(x, w_qkv, w_proj, b_proj, ln1_g, ln1_b, ln2_g, ln2_b,
                 ls1_g, ls2_g, w_fc1, b_fc1, w_fc2, b_fc2):
    x = np.asarray(x, np.float32)
    scale = HD ** -0.5
    g1 = np.asarray(ln1_g, np.float32)
    b1 = np.asarray(ln1_b, np.float32)
    g2 = np.asarray(ln2_g, np.float32)
    b2 = np.asarray(ln2_b, np.float32)
    wqkv = np.array(w_qkv, np.float32, copy=True)
    wqkv[:, :C] *= scale                       # fold q scaling
    qkb = b1 @ wqkv                            # ln1 beta contribution [3C]
    wqkv_g = wqkv * g1[:, None]                # fold ln1 gamma

    def wfmt(w, kdim):
        # [K, M] fp32 -> [128, K/128, M] fp8 host-scaled, partition-major
        kt = kdim // 128
        return np.ascontiguousarray(
            (w * WS).reshape(kt, 128, -1).transpose(1, 0, 2).astype(FP8_NP))

    wproj = (np.asarray(w_proj, np.float32)
             * np.asarray(ls1_g, np.float32)[None, :])
    wfc1_raw = np.asarray(w_fc1, np.float32)
    fcb = b2 @ wfc1_raw                        # ln2 beta contribution [MLP]
    wfc1_g = wfc1_raw * g2[:, None]            # fold ln2 gamma
    wfc2 = (np.asarray(w_fc2, np.float32)
            * np.asarray(ls2_g, np.float32)[None, :])
    common = {
        "wqkv": wfmt(wqkv_g, C), "wproj": wfmt(wproj, C),
        "wfc1": wfmt(wfc1_g, C), "wfc2": wfmt(wfc2, MLP),
        "bls1": _feat_cols(np.asarray(b_proj, np.float32)
                           * np.asarray(ls1_g, np.float32) * WS),
        "bls2": _feat_cols(np.asarray(b_fc2, np.float32)
                           * np.asarray(ls2_g, np.float32) * WS),
        "bfc1": np.ascontiguousarray(
            (np.asarray(b_fc1, np.float32) + fcb).reshape(VT, 128).T),
        "qkvb": _feat_cols(qkb[:2 * C]),
        "vb": np.ascontiguousarray((qkb[2 * C:] * WS)[None, :].astype(BF16_NP)),
    }
    in_maps = []
    for i in range(NCORES):
        xc = x[i * BPC:(i + 1) * BPC]                      # [BPC, N, C]
        xT = np.moveaxis(xc, 2, 0).reshape(C, T)           # [C, T]
        m = dict(common)
        m["xT"] = np.ascontiguousarray(
            (xT * WS).reshape(FT, 128, T).transpose(1, 0, 2).astype(BF16_NP))
        in_maps.append(m)
    return in_maps


def unpack_outputs(results):
    out = np.empty((B, N, C), np.float32)
    for i in range(NCORES):
        oT = results[i]["outT"].transpose(1, 0, 2).reshape(C, T) * WSI
        out[i * BPC:(i + 1) * BPC] = oT.reshape(C, BPC, N).transpose(1, 2, 0)
    return out


def kernel(**inputs):
    nc = _build_program()
    in_maps = make_in_maps(**inputs)
    res = run_bass_kernel_spmd(nc, in_maps, list(range(NCORES)))
    return unpack_outputs(res.results)


if __name__ == "__main__":
    nc = _build_program()
    n_inst = sum(len(bb.instructions) for bb in nc.m.functions[0].blocks)
    print("program built OK, instructions:", n_inst)
